# revision 1
# baseline (speedup 1.0000x reference)
"""GPT forward pass on 8 Trainium2 NeuronCores.

Sharding (Megatron-SP style, feature-major activations):
  - Residual stream x kept TRANSPOSED [D, T_local] and sequence-sharded
    (T=2048 tokens -> 256 per core), SBUF-resident.
  - Attention: tensor-parallel over heads (2 heads/core). AllGather the
    LN output over tokens, compute Q/K feature-major + V token-major,
    causal softmax without max-subtraction (scores are small here and
    softmax is shift-invariant), AllToAll back to sequence sharding.
  - MLP: column-parallel up (512 DFF rows/core), AllGather h, then
    column-parallel down (128 output features/core), AllToAll back.
  - Logits: vocab-sharded (4000 cols/core) over AllGathered final LN.
  - All matmuls in float32r (full PE rate, ~1e-4 matmul error).
"""

import numpy as np

import concourse.bass as bass
import concourse.mybir as mybir
import concourse.tile as tile
from concourse import bacc
from concourse.bass_utils import run_bass_kernel_spmd
from concourse.masks import make_identity

NCORES = 8
V, L, D, NB, H = 32000, 1024, 1024, 8, 16
HD = D // H
DFF = 4 * D
B, S = 2, 1024
T = B * S
TL = T // NCORES  # 256 local tokens
DC = D // 128  # 8 feature chunks
VS = V // NCORES  # 4000 vocab cols per core
VW = 250  # vocab tile width
NVT = VS // VW  # 16
EPS = 1e-5

F32 = mybir.dt.float32
F32R = mybir.dt.float32r
BF16 = mybir.dt.bfloat16
I32 = mybir.dt.int32
AF = mybir.ActivationFunctionType
OP = mybir.AluOpType
RG = [list(range(NCORES))]

MIXED = True

_CACHE: dict = {}


def _build(debug_taps: bool = False, iters: int = 1, mixed: bool = MIXED):
    CT = BF16 if mixed else F32R  # comm/activation dtype
    WT = BF16 if mixed else F32R  # weight dtype
    nc = bacc.Bacc("TRN2", num_devices=NCORES)

    # ---------------- inputs (per-core shards, host-prepped) ----------------
    d_idx = nc.dram_tensor("idx", [TL, 1], I32, kind="ExternalInput")
    d_tok = nc.dram_tensor("tok", [V, D], F32, kind="ExternalInput")
    d_pos = nc.dram_tensor("pos", [2, 128, D], F32, kind="ExternalInput")
    d_wqkv = nc.dram_tensor("wqkv", [NB, D, 384], WT, kind="ExternalInput")
    d_upw = nc.dram_tensor("upw", [NB, D, 512], WT, kind="ExternalInput")
    d_upb = nc.dram_tensor("upb", [NB, 128, 4], F32, kind="ExternalInput")
    d_dww = nc.dram_tensor("dww", [NB, DFF, 128], WT, kind="ExternalInput")
    d_dwb = nc.dram_tensor("dwb", [NB, 128, 1], F32, kind="ExternalInput")
    d_ln1w = nc.dram_tensor("ln1w", [NB, 128, DC], F32, kind="ExternalInput")
    d_ln1b = nc.dram_tensor("ln1b", [NB, 128, DC], F32, kind="ExternalInput")
    d_ln2w = nc.dram_tensor("ln2w", [NB, 128, DC], F32, kind="ExternalInput")
    d_ln2b = nc.dram_tensor("ln2b", [NB, 128, DC], F32, kind="ExternalInput")
    d_lnfw = nc.dram_tensor("lnfw", [128, DC], F32, kind="ExternalInput")
    d_lnfb = nc.dram_tensor("lnfb", [128, DC], F32, kind="ExternalInput")
    d_outw = nc.dram_tensor("outw", [D, VS], WT, kind="ExternalInput")
    d_outb = nc.dram_tensor("outb", [1, VS], F32R, kind="ExternalInput")
    d_ones = nc.dram_tensor("ones", [128, 1024], F32R, kind="ExternalInput")
    d_onesb = nc.dram_tensor("onesb", [128, 32], CT, kind="ExternalInput")
    d_mask = nc.dram_tensor("mask", [128, 4, 512], CT, kind="ExternalInput")

    d_logits = nc.dram_tensor("logits", [T, VS], F32, kind="ExternalOutput")
    taps = {}
    if debug_taps:
        for name in ["x0", "xl1", "xattn", "x1", "xf"]:
            taps[name] = nc.dram_tensor(
                f"tap_{name}", [D, TL], F32, kind="ExternalOutput"
            )

    with nc.allow_low_precision(reason="float32r is intended throughout"), tile.TileContext(nc) as tc:
        with (
            tc.tile_pool(name="const", bufs=1) as constp,
            tc.tile_pool(name="xp", bufs=1) as xp,
            tc.tile_pool(name="wp", bufs=1) as wp,
            tc.tile_pool(name="ap", bufs=2) as ap,
            tc.tile_pool(name="qkv", bufs=1) as qkvp,
            tc.tile_pool(name="row", bufs=1) as rowp,
            tc.tile_pool(name="psmm", bufs=3, space="PSUM") as psmm,
            tc.tile_pool(name="pspv", bufs=2, space="PSUM") as pspv,
            tc.tile_pool(name="psv", bufs=1, space="PSUM") as psv,
            tc.tile_pool(name="psbc", bufs=1, space="PSUM") as psbc,
            tc.tile_pool(name="psst", bufs=1, space="PSUM") as psst,
            tc.tile_pool(name="dram", bufs=1, space="DRAM") as dramp,
        ):
            ones = constp.tile([128, 1024], F32R)
            nc.sync.dma_start(ones[:], d_ones[:])
            ident = constp.tile([128, 128], F32)
            make_identity(nc, ident)
            masks = constp.tile([128, 4, 512], CT)
            nc.sync.dma_start(masks[:], d_mask[:])
            lnfw_t = constp.tile([128, DC], F32)
            nc.sync.dma_start(lnfw_t[:], d_lnfw[:])
            lnfb_t = constp.tile([128, DC], F32)
            nc.sync.dma_start(lnfb_t[:], d_lnfb[:])

            ones_col = ones[:, 0:1]  # [128,1] lhsT for partition sums

            for _it in range(iters):
                x = xp.tile([128, DC, TL], F32R, name="x_resid")

                def tap(name):
                    if debug_taps:
                        nc.sync.dma_start(
                            taps[name].rearrange("(c p) t -> p c t", p=128),
                            x[:].bitcast(F32),
                        )

                # ---------------- embedding ----------------
                emb = ap.tile([128, 2, D], F32, tag="xrhs")
                idx_t = rowp.tile([128, 2], I32, tag="idx", bufs=1)
                nc.sync.dma_start(idx_t[:], d_idx.rearrange("(c p) o -> p (c o)", p=128))
                for c in range(2):
                    nc.gpsimd.indirect_dma_start(
                        emb[:, c, :],
                        None,
                        d_tok[:],
                        bass.IndirectOffsetOnAxis(ap=idx_t[:, c : c + 1], axis=0),
                    )
                    for hf in range(2):
                        pos_t = ap.tile([128, 512], F32, tag="exp")
                        nc.sync.dma_start(pos_t[:], d_pos[c, :, 512 * hf : 512 * (hf + 1)])
                        nc.vector.tensor_add(
                            emb[:, c, 512 * hf : 512 * (hf + 1)],
                            emb[:, c, 512 * hf : 512 * (hf + 1)],
                            pos_t[:],
                        )
                for c in range(2):
                    for dc in range(DC):
                        tp = psv.tile([128, 128], F32, tag="tp")
                        nc.tensor.transpose(
                            tp[:], emb[:, c, dc * 128 : (dc + 1) * 128], ident[:]
                        )
                        nc.vector.tensor_copy(x[:, dc, c * 128 : (c + 1) * 128], tp[:])
                tap("x0")

                # ---------------- helpers ----------------
                def layernorm_to(dst_dram, w_t, b_t):
                    """LN over feature axis of x [128, DC, TL] -> dst_dram [D, TL] f32r."""
                    ps_st = psst.tile([1, 2 * TL], F32, tag="st")
                    for dc in range(DC):
                        nc.tensor.matmul(
                            ps_st[:, 0:TL], ones_col, x[:, dc, :],
                            start=(dc == 0), stop=(dc == DC - 1),
                        )
                    for dc in range(DC):
                        sq = ap.tile([128, TL], F32R, tag="lnsq")
                        nc.vector.tensor_mul(sq[:], x[:, dc, :], x[:, dc, :])
                        nc.tensor.matmul(
                            ps_st[:, TL : 2 * TL], ones_col, sq[:],
                            start=(dc == 0), stop=(dc == DC - 1),
                        )
                    mu = rowp.tile([1, TL], F32, tag="mu")
                    nc.vector.tensor_scalar(
                        out=mu[:], in0=ps_st[:, 0:TL], scalar1=1.0 / D, scalar2=None,
                        op0=OP.mult,
                    )
                    var = rowp.tile([1, TL], F32, tag="var")
                    nc.vector.tensor_scalar(
                        out=var[:], in0=ps_st[:, TL : 2 * TL], scalar1=1.0 / D,
                        scalar2=EPS, op0=OP.mult, op1=OP.add,
                    )
                    mu2 = rowp.tile([1, TL], F32, tag="mu2")
                    nc.vector.tensor_mul(mu2[:], mu[:], mu[:])
                    nc.vector.tensor_sub(var[:], var[:], mu2[:])
                    row = rowp.tile([1, 2 * TL], F32R, tag="row")
                    rootv = rowp.tile([1, TL], F32, tag="rootv")
                    nc.scalar.activation(rootv[:], var[:], AF.Sqrt)
                    nc.vector.reciprocal(row[:, 0:TL], rootv[:])
                    negmu = rowp.tile([1, TL], F32, tag="negmu")
                    nc.vector.tensor_scalar(
                        out=negmu[:], in0=mu[:], scalar1=-1.0, scalar2=None, op0=OP.mult,
                    )
                    nc.vector.tensor_mul(row[:, TL : 2 * TL], negmu[:], row[:, 0:TL])
                    ps_bc = psbc.tile([128, 512], F32, tag="bc")
                    nc.tensor.matmul(
                        ps_bc[:, 0 : 2 * TL], ones[0:1, 0:128], row[:],
                        start=True, stop=True,
                    )
                    dst_r = dst_dram.rearrange("(c p) t -> p c t", p=128)
                    for dc in range(DC):
                        t1 = ap.tile([128, TL], F32, tag="ln_t1")
                        nc.vector.tensor_mul(
                            t1[:], x[:, dc, :].bitcast(F32), ps_bc[:, 0:TL]
                        )
                        nc.vector.tensor_add(t1[:], t1[:], ps_bc[:, TL : 2 * TL])
                        o = ap.tile([128, TL], CT, tag="ln_o")
                        nc.scalar.activation(
                            o[:], t1[:], AF.Identity,
                            bias=b_t[:, dc : dc + 1], scale=w_t[:, dc : dc + 1],
                        )
                        nc.sync.dma_start(dst_r[:, dc, :], o[:])

                def resid_add(a2a_out):
                    src = a2a_out.rearrange("(c p) t -> p c t", p=128)
                    for dc in range(DC):
                        tmp = ap.tile([128, TL], CT, tag="resid")
                        nc.sync.dma_start(tmp[:], src[:, dc, :])
                        nc.vector.tensor_add(x[:, dc, :], x[:, dc, :], tmp[:])

                def ki_list(qj):
                    out = []
                    for ki in range(8):
                        if 128 * ki + 127 <= 512 * qj:
                            out.append((ki, None))  # full
                        elif 128 * ki <= 512 * qj + 511:
                            out.append((ki, ki - 4 * qj))  # diagonal
                    return out

                # ---------------- layers ----------------
                for li in range(NB):
                    ln1w_t = wp.tile([128, DC], F32, tag="ln1w")
                    nc.sync.dma_start(ln1w_t[:], d_ln1w[li])
                    ln1b_t = wp.tile([128, DC], F32, tag="ln1b")
                    nc.sync.dma_start(ln1b_t[:], d_ln1b[li])
                    ag1_in = dramp.tile([D, TL], CT, tag="ag_in", bufs=2)
                    layernorm_to(ag1_in, ln1w_t, ln1b_t)
                    if li == 0 and debug_taps and not mixed:
                        nc.sync.dma_start(taps["xl1"][:], ag1_in[:].bitcast(F32))
                    ag1_out = dramp.tile(
                        [NCORES * D, TL], CT, tag="ag_out", bufs=2, addr_space="Shared"
                    )
                    nc.gpsimd.collective_compute(
                        "AllGather", OP.bypass, replica_groups=RG,
                        ins=[ag1_in[:]], outs=[ag1_out[:]],
                    )
                    agv = ag1_out.rearrange("(j c p) t -> c p j t", j=NCORES, p=128)

                    wqkv_t = wp.tile([128, DC, 384], WT, tag="wqkv")
                    nc.sync.dma_start(
                        wqkv_t[:], d_wqkv[li].rearrange("(c p) m -> p c m", p=128)
                    )
                    a2a_in = dramp.tile([NCORES * 128, TL], CT, tag="a2a_in", bufs=2)
                    a2a_in_v = a2a_in.rearrange("(j p) t -> p j t", p=128)

                    for b in range(B):
                        # --- QKV for batch b ---
                        q_T = qkvp.tile([128, S], CT, tag="qT")
                        k_T = qkvp.tile([128, S], CT, tag="kT")
                        v_n = qkvp.tile([128, 8, 130], CT, tag="vn")
                        nc.sync.dma_start(v_n[:, :, 64:65], d_onesb[:, 0:8].unsqueeze(2))
                        nc.sync.dma_start(v_n[:, :, 129:130], d_onesb[:, 8:16].unsqueeze(2))
                        for tt in range(2):  # 512-token spans within batch
                            j0 = 4 * b + 2 * tt
                            xr = ap.tile([128, DC, 512], CT, tag="xrhs")
                            for dc in range(DC):
                                nc.sync.dma_start(xr[:, dc, :], agv[dc, :, j0 : j0 + 2, :])
                            ps_q = psmm.tile([128, 512], F32, tag="mm")
                            ps_k = psmm.tile([128, 512], F32, tag="mm")
                            for dc in range(DC):
                                nc.tensor.matmul(
                                    ps_q[:], wqkv_t[:, dc, 0:128], xr[:, dc, :],
                                    start=(dc == 0), stop=(dc == DC - 1),
                                )
                            for dc in range(DC):
                                nc.tensor.matmul(
                                    ps_k[:], wqkv_t[:, dc, 128:256], xr[:, dc, :],
                                    start=(dc == 0), stop=(dc == DC - 1),
                                )
                            nc.vector.tensor_copy(
                                q_T[:, 512 * tt : 512 * (tt + 1)], ps_q[:]
                            )
                            nc.vector.tensor_copy(
                                k_T[:, 512 * tt : 512 * (tt + 1)], ps_k[:]
                            )
                            for mt in range(4):
                                ps_vt = psv.tile([128, 128], F32, tag="tp")
                                for dc in range(DC):
                                    nc.tensor.matmul(
                                        ps_vt[:],
                                        xr[:, dc, mt * 128 : (mt + 1) * 128],
                                        wqkv_t[:, dc, 256:384],
                                        start=(dc == 0), stop=(dc == DC - 1),
                                    )
                                ki = tt * 4 + mt
                                nc.scalar.activation(
                                    v_n[:, ki, 0:64], ps_vt[:, 0:64], AF.Copy
                                )
                                nc.scalar.activation(
                                    v_n[:, ki, 65:129], ps_vt[:, 64:128], AF.Copy
                                )
                        # --- attention for batch b ---
                        for hh in range(2):
                            hp = 64 * hh
                            vc0 = 65 * hh
                            for qj in range(2):
                                q_sl = q_T[hp : hp + 64, 512 * qj : 512 * (qj + 1)]
                                ps_pv = pspv.tile([65, 512], F32, tag="pv")
                                kis = ki_list(qj)
                                for en, (ki, mj) in enumerate(kis):
                                    ps_s = psmm.tile([128, 512], F32, tag="mm")
                                    nc.tensor.matmul(
                                        ps_s[:],
                                        k_T[hp : hp + 64, 128 * ki : 128 * (ki + 1)],
                                        q_sl,
                                        start=True, stop=True,
                                    )
                                    ex = ap.tile([128, 512], CT, tag="exp")
                                    nc.scalar.activation(
                                        ex[:], ps_s[:], AF.Exp, scale=0.125
                                    )
                                    if mj is not None:
                                        nc.vector.tensor_mul(
                                            ex[:], ex[:], masks[:, mj, :]
                                        )
                                    nc.tensor.matmul(
                                        ps_pv[:],
                                        v_n[:, ki, vc0 : vc0 + 65],
                                        ex[:],
                                        start=(en == 0), stop=(en == len(kis) - 1),
                                    )
                                recip = rowp.tile([1, 512], F32R, tag="recip")
                                nc.vector.reciprocal(recip[:], ps_pv[64:65, :])
                                ps_r = psbc.tile([128, 512], F32, tag="bc")
                                nc.tensor.matmul(
                                    ps_r[0:64, :], ones[0:1, 0:64], recip[:],
                                    start=True, stop=True,
                                )
                                au = ap.tile([64, 512], F32, tag="au")
                                nc.scalar.activation(au[:], ps_pv[0:64, :], AF.Copy)
                                asl = ap.tile([64, 512], CT, tag="asl")
                                nc.vector.tensor_mul(asl[:], au[:], ps_r[0:64, :])
                                j0 = 4 * b + 2 * qj
                                nc.sync.dma_start(
                                    a2a_in_v[hp : hp + 64, j0 : j0 + 2, :], asl[:]
                                )
                    a2a_out = dramp.tile([NCORES * 128, TL], CT, tag="a2a_out", bufs=2)
                    nc.gpsimd.collective_compute(
                        "AllToAll", OP.bypass, replica_groups=RG,
                        ins=[a2a_in[:]], outs=[a2a_out[:]],
                    )
                    resid_add(a2a_out)
                    if li == 0:
                        tap("xattn")

                    # --- MLP ---
                    ln2w_t = wp.tile([128, DC], F32, tag="ln2w")
                    nc.sync.dma_start(ln2w_t[:], d_ln2w[li])
                    ln2b_t = wp.tile([128, DC], F32, tag="ln2b")
                    nc.sync.dma_start(ln2b_t[:], d_ln2b[li])
                    ag2_in = dramp.tile([D, TL], CT, tag="ag_in", bufs=2)
                    layernorm_to(ag2_in, ln2w_t, ln2b_t)
                    ag2_out = dramp.tile(
                        [NCORES * D, TL], CT, tag="ag_out", bufs=2, addr_space="Shared"
                    )
                    nc.gpsimd.collective_compute(
                        "AllGather", OP.bypass, replica_groups=RG,
                        ins=[ag2_in[:]], outs=[ag2_out[:]],
                    )
                    ag2v = ag2_out.rearrange("(j c p) t -> c p j t", j=NCORES, p=128)

                    upw_t = wp.tile([128, DC, 512], WT, tag="upw")
                    nc.sync.dma_start(
                        upw_t[:], d_upw[li].rearrange("(c p) m -> p c m", p=128)
                    )
                    upb_t = wp.tile([128, 4], F32, tag="upb")
                    nc.sync.dma_start(upb_t[:], d_upb[li])
                    h_in = dramp.tile([2, 512, S], CT, tag="h_in", bufs=2)
                    h_outs = []
                    for nt in range(4):
                        xr2 = ap.tile([128, DC, 512], CT, tag="xrhs")
                        for dc in range(DC):
                            nc.sync.dma_start(
                                xr2[:, dc, :], ag2v[dc, :, 2 * nt : 2 * nt + 2, :]
                            )
                        for mt in range(4):
                            ps_u = psmm.tile([128, 512], F32, tag="mm")
                            for dc in range(DC):
                                nc.tensor.matmul(
                                    ps_u[:],
                                    upw_t[:, dc, mt * 128 : (mt + 1) * 128],
                                    xr2[:, dc, :],
                                    start=(dc == 0), stop=(dc == DC - 1),
                                )
                            hsb = ap.tile([128, 512], CT, tag="hsb")
                            nc.scalar.activation(
                                hsb[:], ps_u[:], AF.Relu, bias=upb_t[:, mt : mt + 1]
                            )
                            nc.sync.dma_start(
                                h_in[
                                    nt // 2,
                                    128 * mt : 128 * (mt + 1),
                                    512 * (nt % 2) : 512 * (nt % 2 + 1),
                                ],
                                hsb[:],
                            )
                        if nt % 2 == 1:
                            ho = dramp.tile(
                                [NCORES * 512, S], CT, tag="h_out", bufs=2,
                                addr_space="Shared",
                            )
                            nc.gpsimd.collective_compute(
                                "AllGather", OP.bypass, replica_groups=RG,
                                ins=[h_in[nt // 2]], outs=[ho[:]],
                            )
                            h_outs.append(ho)

                    dww_t = wp.tile([128, 32, 128], WT, tag="dww")
                    nc.sync.dma_start(
                        dww_t[:], d_dww[li].rearrange("(c p) m -> p c m", p=128)
                    )
                    dwb_t = wp.tile([128, 1], F32, tag="dwb")
                    nc.sync.dma_start(dwb_t[:], d_dwb[li])
                    a2m_in = dramp.tile([NCORES * 128, TL], CT, tag="a2a_in", bufs=2)
                    a2m_in_v = a2m_in.rearrange("(j p) t -> p j t", p=128)
                    for nt in range(4):
                        ps_d = psmm.tile([128, 512], F32, tag="mm")
                        ho = h_outs[nt // 2]
                        for kc in range(32):
                            hr = ap.tile([128, 512], CT, tag="hrhs", bufs=3)
                            nc.sync.dma_start(
                                hr[:],
                                ho[
                                    128 * kc : 128 * (kc + 1),
                                    512 * (nt % 2) : 512 * (nt % 2 + 1),
                                ],
                            )
                            nc.tensor.matmul(
                                ps_d[:], dww_t[:, kc, :], hr[:],
                                start=(kc == 0), stop=(kc == 31),
                            )
                        ysb = ap.tile([128, 512], CT, tag="ysb")
                        nc.scalar.activation(ysb[:], ps_d[:], AF.Identity, bias=dwb_t[:])
                        nc.sync.dma_start(a2m_in_v[:, 2 * nt : 2 * nt + 2, :], ysb[:])
                    a2m_out = dramp.tile([NCORES * 128, TL], CT, tag="a2a_out", bufs=2)
                    nc.gpsimd.collective_compute(
                        "AllToAll", OP.bypass, replica_groups=RG,
                        ins=[a2m_in[:]], outs=[a2m_out[:]],
                    )
                    resid_add(a2m_out)
                    if li == 0:
                        tap("x1")

                # ---------------- final LN + logits ----------------
                tap("xf")
                agf_in = dramp.tile([D, TL], CT, tag="ag_in", bufs=2)
                layernorm_to(agf_in, lnfw_t, lnfb_t)
                agf_out = dramp.tile(
                    [NCORES * D, TL], CT, tag="ag_out", bufs=2, addr_space="Shared"
                )
                nc.gpsimd.collective_compute(
                    "AllGather", OP.bypass, replica_groups=RG,
                    ins=[agf_in[:]], outs=[agf_out[:]],
                )
                agfv = agf_out.rearrange("(j c p) t -> c p j t", j=NCORES, p=128)
                oww = d_outw.rearrange("(c p) n -> p c n", p=128)
                for mtg in range(2):  # 1024-token groups
                    xfc = ap.tile([128, DC, 1024], CT, tag="xfc", bufs=1)
                    for dc in range(DC):
                        nc.sync.dma_start(
                            xfc[:, dc, :], agfv[dc, :, 4 * mtg : 4 * mtg + 4, :]
                        )
                    for vt in range(NVT):
                        ow = ap.tile([128, DC, VW], WT, tag="ow")
                        nc.sync.dma_start(ow[:], oww[:, :, VW * vt : VW * (vt + 1)])
                        obr = rowp.tile([1, VW], F32R, tag="obr")
                        nc.sync.dma_start(obr[:], d_outb[:, VW * vt : VW * (vt + 1)])
                        ps_b = psbc.tile([128, 512], F32, tag="bc")
                        nc.tensor.matmul(
                            ps_b[:, 0:VW], ones[0:1, 0:128], obr[:], start=True, stop=True
                        )
                        bias_sb = ap.tile([128, VW], F32, tag="lbias")
                        nc.vector.tensor_copy(bias_sb[:], ps_b[:, 0:VW])
                        for mt in range(8):
                            ps_l = psmm.tile([128, 512], F32, tag="mm")
                            for dc in range(DC):
                                nc.tensor.matmul(
                                    ps_l[:, 0:VW],
                                    xfc[:, dc, 128 * mt : 128 * (mt + 1)],
                                    ow[:, dc, :],
                                    start=(dc == 0), stop=(dc == DC - 1),
                                )
                            lo = ap.tile([128, VW], F32, tag="lo", bufs=3)
                            nc.vector.tensor_add(lo[:], ps_l[:, 0:VW], bias_sb[:])
                            nc.sync.dma_start(
                                d_logits[
                                    1024 * mtg + 128 * mt : 1024 * mtg + 128 * (mt + 1),
                                    VW * vt : VW * (vt + 1),
                                ],
                                lo[:],
                            )

    nc.finalize()
    return nc


def _prep_inputs(inputs) -> list[dict]:
    tok_emb = np.ascontiguousarray(np.asarray(inputs["tok_emb"], dtype=np.float32))
    pos_emb = np.asarray(inputs["pos_emb"], dtype=np.float32)
    ctx = np.asarray(inputs["context"]).astype(np.int32).reshape(-1)  # [T]
    f32 = lambda k: np.asarray(inputs[k], dtype=np.float32)
    wq, wk, wv = f32("wq"), f32("wk"), f32("wv")
    up_w, up_b = f32("up_w"), f32("up_b")
    down_w, down_b = f32("down_w"), f32("down_b")
    ln1_w, ln1_b = f32("ln1_w"), f32("ln1_b")
    ln2_w, ln2_b = f32("ln2_w"), f32("ln2_b")
    lnf_w, lnf_b = f32("lnf_w"), f32("lnf_b")
    out_w, out_b = f32("out_w"), f32("out_b")

    import ml_dtypes

    wdt = ml_dtypes.bfloat16 if MIXED else np.float32
    ones = np.ones((128, 1024), np.float32)
    ones[:, 512:] = -1.0
    onesb = np.ones((128, 32), wdt)
    mask = np.zeros((128, 4, 512), np.float32)
    for j in range(4):
        for p in range(128):
            mask[p, j, 128 * j + p :] = 1.0

    def ln_pack(w):  # [D] -> [128, DC]
        return np.ascontiguousarray(w.reshape(DC, 128).T)

    in_maps = []
    for r in range(NCORES):
        tl = np.arange(TL) + r * TL
        s_pos = tl % S
        wqkv = np.concatenate(
            [
                wq[:, 128 * r : 128 * (r + 1), :].transpose(0, 2, 1),
                wk[:, 128 * r : 128 * (r + 1), :].transpose(0, 2, 1),
                wv[:, 128 * r : 128 * (r + 1), :].transpose(0, 2, 1),
            ],
            axis=2,
        )  # [NB, D, 384]
        m = {
            "idx": ctx[tl][:, None].astype(np.int32),
            "tok": tok_emb,
            "pos": np.ascontiguousarray(pos_emb[s_pos].reshape(2, 128, D)),
            "wqkv": np.ascontiguousarray(wqkv.astype(wdt)),
            "upw": np.ascontiguousarray(
                up_w[:, 512 * r : 512 * (r + 1), :].transpose(0, 2, 1).astype(wdt)
            ),
            "upb": np.ascontiguousarray(
                up_b[:, 512 * r : 512 * (r + 1)].reshape(NB, 4, 128).transpose(0, 2, 1)
            ),
            "dww": np.ascontiguousarray(
                down_w[:, 128 * r : 128 * (r + 1), :].transpose(0, 2, 1).astype(wdt)
            ),
            "dwb": np.ascontiguousarray(
                down_b[:, 128 * r : 128 * (r + 1)][:, :, None]
            ),
            "ln1w": np.stack([ln_pack(ln1_w[i]) for i in range(NB)]),
            "ln1b": np.stack([ln_pack(ln1_b[i]) for i in range(NB)]),
            "ln2w": np.stack([ln_pack(ln2_w[i]) for i in range(NB)]),
            "ln2b": np.stack([ln_pack(ln2_b[i]) for i in range(NB)]),
            "lnfw": ln_pack(lnf_w),
            "lnfb": ln_pack(lnf_b),
            "outw": np.ascontiguousarray(out_w[VS * r : VS * (r + 1), :].T.astype(wdt)),
            "outb": np.ascontiguousarray(out_b[VS * r : VS * (r + 1)][None, :]),
            "ones": ones,
            "onesb": onesb,
            "mask": mask.astype(wdt),
        }
        in_maps.append(m)
    return in_maps


def kernel(**inputs) -> np.ndarray:
    if "nc" not in _CACHE:
        _CACHE["nc"] = _build()
    nc = _CACHE["nc"]
    in_maps = _prep_inputs(inputs)
    res = run_bass_kernel_spmd(nc, in_maps, list(range(NCORES))).results
    logits = np.concatenate([res[r]["logits"] for r in range(NCORES)], axis=1)
    return logits.reshape(B, S, V).astype(np.float32)

